# revision 1
# baseline (speedup 1.0000x reference)
"""Tensor-parallel TinyLlama prefill decoder on 8 Trainium2 NeuronCores.

Returns the stacked pre-RoPE KV cache [2, L, B, H, S, HD] (the only live
output of the reference's prefill forward; the final layer's attention/FFN
are dead code and are skipped).

Sharding: tensor-parallel over heads (2/core) and FFN columns (704/core);
norms replicated. Two fp16 AllReduces per layer (attention out + FFN down
partials). Activations live transposed ([E, S]) in SBUF so every matmul
contracts along partitions without transposes; scores are computed
transposed ([k, q]) so the softmax denominator falls out of the o-matmul
via an appended ones column on v.
"""

import os
from contextlib import ExitStack

import numpy as np

import concourse.bass as bass
import concourse.mybir as mybir
import concourse.tile as tile
from concourse import bacc
from concourse.bass_utils import run_bass_kernel_spmd

F16 = mybir.dt.float16
F32 = mybir.dt.float32
AF = mybir.ActivationFunctionType

# model config (hardcoded per contract)
B, S, E, H, HD, FF, L, V = 1, 2048, 2048, 16, 128, 5632, 4, 32000
ROPE_THETA = 10000.0
EPS = 1e-5
NC = 8                      # cores
HPC = H // NC               # heads per core (2)
DPC = HPC * HD              # qkv dims per core (256)
FPC = FF // NC              # ffn dims per core (704)
ET = E // 128               # E tiles (16)
ST = S // 128               # S blocks (16)
SC = 512                    # matmul free-dim chunk
NSC = S // SC               # chunks over S (4)
FT = 5                      # full 128-row FF tiles; plus one 64-row tile
SCALE = float(HD) ** -0.5

_CACHE = {}


def build_kernel():
    nc = bacc.Bacc("TRN2", target_bir_lowering=False, debug=False,
                   num_devices=NC)

    # ---- DRAM I/O --------------------------------------------------------
    x0T = nc.dram_tensor("x0T", [E, S], F16, kind="ExternalInput").ap()
    wq = nc.dram_tensor("wq", [L, E, DPC], F16, kind="ExternalInput").ap()
    wk = nc.dram_tensor("wk", [L, E, DPC], F16, kind="ExternalInput").ap()
    wv = nc.dram_tensor("wv", [L, E, DPC], F16, kind="ExternalInput").ap()
    wo = nc.dram_tensor("wo", [L, DPC, E], F16, kind="ExternalInput").ap()
    wg = nc.dram_tensor("wg", [L, E, FPC], F16, kind="ExternalInput").ap()
    wu = nc.dram_tensor("wu", [L, E, FPC], F16, kind="ExternalInput").ap()
    wd = nc.dram_tensor("wd", [L, FPC, E], F16, kind="ExternalInput").ap()
    cosT = nc.dram_tensor("cosT", [HD, S], F16, kind="ExternalInput").ap()
    sinT = nc.dram_tensor("sinT", [HD, S], F16, kind="ExternalInput").ap()
    rotP = nc.dram_tensor("rotP", [HD, HD], F16, kind="ExternalInput").ap()
    triM = nc.dram_tensor("triM", [128, 128], F16, kind="ExternalInput").ap()
    idnt = nc.dram_tensor("idnt", [128, 128], F16, kind="ExternalInput").ap()
    kvo = nc.dram_tensor("kv_out", [2, L, HPC, S, HD], F16,
                         kind="ExternalOutput").ap()

    with tile.TileContext(nc) as tc, ExitStack() as ctx:
        ctx.enter_context(nc.allow_low_precision(
            reason="fp16 kernel by design; accumulation stays fp32 in PSUM"))

        # ---- persistent SBUF ---------------------------------------------
        px = ctx.enter_context(tc.tile_pool(name="px", bufs=ET))
        x_t = []
        for e in range(ET):
            t = px.tile([128, S], F16, name=f"x_{e}", tag="x")
            nc.sync.dma_start(t[:], x0T[e * 128:(e + 1) * 128, :])
            x_t.append(t)

        pc = ctx.enter_context(tc.tile_pool(name="pconst", bufs=1))
        cos_sb = pc.tile([HD, S], F16, name="cos_sb")
        sin_sb = pc.tile([HD, S], F16, name="sin_sb")
        rot_sb = pc.tile([HD, HD], F16, name="rot_sb")
        tri_sb = pc.tile([128, 128], F16, name="tri_sb")
        id_sb = pc.tile([128, 128], F16, name="id_sb")
        ones_sb = pc.tile([128, 128], F16, name="ones_sb")
        eps_sb = pc.tile([128, 1], F32, name="eps_sb")
        nc.sync.dma_start(cos_sb[:], cosT[:])
        nc.sync.dma_start(sin_sb[:], sinT[:])
        nc.sync.dma_start(rot_sb[:], rotP[:])
        nc.sync.dma_start(tri_sb[:], triM[:])
        nc.sync.dma_start(id_sb[:], idnt[:])
        nc.gpsimd.memset(ones_sb[:], 1.0)
        nc.gpsimd.memset(eps_sb[:], EPS)

        # DRAM bounce buffers for the AllReduces
        pdram = ctx.enter_context(tc.tile_pool(name="pdram", bufs=1,
                                               space="DRAM"))
        ar_in = pdram.tile([E, S], F16, name="ar_in")
        # Shared DRAM outputs are single-writer: one tile per AllReduce
        ar_outs = [pdram.tile([E, S], F16, name=f"ar_out{i}",
                              addr_space="Shared", tag=f"ar_out{i}")
                   for i in range(2 * (L - 1))]

        # rotating work pools (SBUF)
        pw = ctx.enter_context(tc.tile_pool(name="pw", bufs=2))
        pn = ctx.enter_context(tc.tile_pool(name="pn", bufs=2))
        pqk = ctx.enter_context(tc.tile_pool(name="pqk", bufs=1))
        pv = ctx.enter_context(tc.tile_pool(name="pv", bufs=18))
        pat = ctx.enter_context(tc.tile_pool(name="pat", bufs=3))
        pff = ctx.enter_context(tc.tile_pool(name="pff", bufs=2))
        pio = ctx.enter_context(tc.tile_pool(name="pio", bufs=2))

        def rms_norm_factors(l, tag):
            """R [128, S] (rows all equal rsqrt(mean(x^2)+eps)) and
            rT [128, ST] (column sb = per-partition r for s-block sb)."""
            with tc.tile_pool(name=f"ps_n_{l}_{tag}", bufs=1,
                              space="PSUM") as psn:
                ss = [psn.tile([128, SC], F32, name=f"ss{i}", tag=f"ss{i}",
                               bufs=1) for i in range(NSC)]
                for e in range(ET):
                    for i in range(NSC):
                        c = slice(i * SC, (i + 1) * SC)
                        x2 = pn.tile([128, SC], F16, name=f"x2_{e}_{i}",
                                     tag="x2", bufs=3)
                        nc.vector.tensor_mul(x2[:], x_t[e][:, c],
                                             x_t[e][:, c])
                        nc.tensor.matmul(ss[i][:], ones_sb[:], x2[:],
                                         start=(e == 0), stop=(e == ET - 1))
                R = pn.tile([128, S], F16, name=f"R_{l}_{tag}", tag="R",
                            bufs=2)
                for i in range(NSC):
                    c = slice(i * SC, (i + 1) * SC)
                    sq = pn.tile([128, SC], F16, name=f"sq{i}", tag="sq",
                                 bufs=2)
                    nc.scalar.activation(sq[:], ss[i][:], AF.Sqrt,
                                         bias=eps_sb[:], scale=1.0 / E)
                    nc.vector.reciprocal(R[:, c], sq[:])
                # transpose R to get per-partition scalars rT
                rT = pn.tile([128, ST], F32, name=f"rT_{l}_{tag}",
                             tag="rT", bufs=2)
                with tc.tile_pool(name=f"ps_rt_{l}_{tag}", bufs=2,
                                  space="PSUM") as psr:
                    for sb in range(ST):
                        tp = psr.tile([128, 128], F16, name=f"tpr{sb}",
                                      tag="tpr")
                        nc.tensor.transpose(
                            tp[:], R[:, sb * 128:(sb + 1) * 128], id_sb[:])
                        nc.vector.tensor_copy(rT[:, sb:sb + 1], tp[:, 0:1])
            return R, rT

        def load_w_cols(dram_ap, cols, name, tag):
            """DRAM [E, cols] -> SBUF [128, ET*cols], E-tile major."""
            t = pw.tile([128, ET * cols], F16, name=name, tag=tag, bufs=2)
            nc.sync.dma_start(
                t[:].rearrange("p (t m) -> p t m", t=ET),
                dram_ap.rearrange("(t p) m -> p t m", p=128))
            return t

        for l in range(L):
            act = l < L - 1
            R1, rT1 = rms_norm_factors(l, "a")

            # ---- q/k projections + RoPE + k output ----------------------
            # raw chunks are transient; rope'd q/k persist per head.
            wk_sb = load_w_cols(wk[l], DPC, f"wk_sb_{l}", "wsm")
            wq_sb = load_w_cols(wq[l], DPC, f"wq_sb_{l}", "wsm") if act \
                else None
            qr_sb, kr_sb = [], []
            with tc.tile_pool(name=f"ps_qk_{l}", bufs=1, space="PSUM") as pq:
                srcs = [("k", wk_sb)] + ([("q", wq_sb)] if act else [])
                for nmw, wsb in srcs:
                    for h in range(HPC):
                        t = None
                        if act:
                            t = pqk.tile([128, S], F16, name=f"{nmw}r_{l}_{h}",
                                         tag=f"{nmw}r{h}", bufs=1)
                        for i in range(NSC):
                            c = slice(i * SC, (i + 1) * SC)
                            ps = pq.tile([128, SC], F32, name=f"qk{i}",
                                         tag="qkps", bufs=3)
                            for e in range(ET):
                                nc.tensor.matmul(
                                    ps[:],
                                    wsb[:, e * DPC + h * 128:
                                        e * DPC + (h + 1) * 128],
                                    x_t[e][:, c],
                                    start=(e == 0), stop=(e == ET - 1))
                            raw = pn.tile([128, SC], F16, name=f"raw{i}",
                                          tag="qkraw", bufs=3)
                            nc.vector.tensor_mul(raw[:], ps[:], R1[:, c])
                            if nmw == "k":
                                # k output (pre-RoPE): [d, s] -> [s, d]
                                for j in range(4):
                                    sb = i * 4 + j
                                    tp = pq.tile([128, 128], F16,
                                                 name=f"ko{sb}", tag="kops",
                                                 bufs=2)
                                    nc.tensor.transpose(
                                        tp[:], raw[:, j * 128:(j + 1) * 128],
                                        id_sb[:])
                                    ko = pio.tile([128, 128], F16,
                                                  name=f"kos_{sb}",
                                                  tag="kosb", bufs=2)
                                    nc.vector.tensor_copy(ko[:], tp[:])
                                    nc.sync.dma_start(
                                        kvo[0, l, h,
                                            sb * 128:(sb + 1) * 128, :],
                                        ko[:])
                            if act:
                                # RoPE: t = raw*cos + (rotP.T @ raw)*sin
                                rp = pq.tile([128, SC], F32, name=f"rot{i}",
                                             tag="rotps", bufs=2)
                                nc.tensor.matmul(rp[:], rot_sb[:], raw[:],
                                                 start=True, stop=True)
                                nc.vector.tensor_mul(t[:, c], raw[:],
                                                     cos_sb[:, c])
                                tmp = pn.tile([128, SC], F16, name=f"rtmp{i}",
                                              tag="rtmp", bufs=2)
                                nc.vector.tensor_mul(tmp[:], rp[:],
                                                     sin_sb[:, c])
                                nc.vector.tensor_add(t[:, c], t[:, c],
                                                     tmp[:])
                        if act:
                            (kr_sb if nmw == "k" else qr_sb).append(t)

            # ---- v in [s, d] layout, normalized, ones col, v output -----
            wv_sb = load_w_cols(wv[l], DPC, f"wv_sb_{l}", "wsm")
            vext = [[None] * ST for _ in range(HPC)]
            with tc.tile_pool(name=f"ps_v_{l}", bufs=3, space="PSUM") as pvv:
                for sb in range(ST):
                    ps = pvv.tile([128, DPC], F32, name=f"v{sb}", tag="vps")
                    for e in range(ET):
                        nc.tensor.matmul(
                            ps[:], x_t[e][:, sb * 128:(sb + 1) * 128],
                            wv_sb[:, e * DPC:(e + 1) * DPC],
                            start=(e == 0), stop=(e == ET - 1))
                    for h in range(HPC):
                        vt = pv.tile([128, 132], F16, name=f"vx_{h}_{sb}",
                                     tag=f"vx{h}")
                        nc.vector.tensor_scalar_mul(
                            vt[:, 0:128], ps[:, h * 128:(h + 1) * 128],
                            rT1[:, sb:sb + 1])
                        nc.gpsimd.memset(vt[:, 128:132], 1.0)
                        vext[h][sb] = vt
                        nc.sync.dma_start(
                            kvo[1, l, h, sb * 128:(sb + 1) * 128, :],
                            vt[:, 0:128])

            if not act:
                continue

            # ---- attention (scores transposed) --------------------------
            oT = []  # per head [128, S] fp16 (o transposed)
            for h in range(HPC):
                ot = pqk.tile([128, S], F16, name=f"oT_{l}_{h}",
                              tag=f"oT{h}", bufs=1)
                with tc.tile_pool(name=f"ps_at_{l}_{h}", bufs=1,
                                  space="PSUM") as pa:
                    for qq in range(NSC):          # 512-wide q groups
                        qc = slice(qq * SC, (qq + 1) * SC)
                        ops = [pa.tile([128, 132], F32, name=f"o{qq}_{j}",
                                       tag=f"ops{j}", bufs=1)
                               for j in range(4)]
                        nkb = 4 * qq + 4
                        for kb in range(nkb):
                            st = pa.tile([128, SC], F32, name=f"st{kb}",
                                         tag="stps", bufs=2)
                            nc.tensor.matmul(
                                st[:],
                                kr_sb[h][:, kb * 128:(kb + 1) * 128],
                                qr_sb[h][:, qc], start=True, stop=True)
                            ex = pat.tile([128, SC], F16, name=f"ex{kb}",
                                          tag="ex")
                            nc.scalar.activation(ex[:], st[:], AF.Exp,
                                                 scale=SCALE)
                            for j in range(4):
                                qb = 4 * qq + j
                                if qb < kb:
                                    continue
                                exs = ex[:, j * 128:(j + 1) * 128]
                                if qb == kb:
                                    nc.vector.tensor_mul(exs, exs, tri_sb[:])
                                nc.tensor.matmul(ops[j][:], exs,
                                                 vext[h][kb][:],
                                                 start=(kb == 0),
                                                 stop=(kb == nkb - 1))
                        for j in range(4):
                            qb = 4 * qq + j
                            rec = pn.tile([128, 1], F32, name=f"rec{j}",
                                          tag="rec", bufs=2)
                            nc.vector.reciprocal(rec[:], ops[j][:, 128:129])
                            ob = pio.tile([128, 128], F16, name=f"ob{qb}",
                                          tag="ob", bufs=2)
                            nc.vector.tensor_scalar_mul(ob[:],
                                                        ops[j][:, 0:128],
                                                        rec[:])
                            tp = pa.tile([128, 128], F16, name=f"to{qb}",
                                         tag="tops", bufs=2)
                            nc.tensor.transpose(tp[:], ob[:], id_sb[:])
                            nc.vector.tensor_copy(
                                ot[:, qb * 128:(qb + 1) * 128], tp[:])
                oT.append(ot)

            # ---- Wo partial + AllReduce + residual ----------------------
            wo_sb = pw.tile([128, HPC * E], F16, name=f"wo_sb_{l}",
                            tag="wsm", bufs=2)
            nc.sync.dma_start(
                wo_sb[:].rearrange("p (t m) -> p t m", t=HPC),
                wo[l].rearrange("(t p) m -> p t m", p=128))
            with tc.tile_pool(name=f"ps_wo_{l}", bufs=4, space="PSUM") as pwo:
                for m in range(ET):
                    for i in range(NSC):
                        ps = pwo.tile([128, SC], F32, name=f"wo{i}",
                                      tag="wops")
                        for h in range(HPC):
                            nc.tensor.matmul(
                                ps[:],
                                wo_sb[:, h * E + m * 128:
                                      h * E + (m + 1) * 128],
                                oT[h][:, i * SC:(i + 1) * SC],
                                start=(h == 0), stop=(h == HPC - 1))
                        cs = pio.tile([128, SC], F16, name=f"woc_{m}_{i}",
                                      tag="wocast", bufs=2)
                        nc.scalar.copy(cs[:], ps[:])
                        nc.sync.dma_start(
                            ar_in[m * 128:(m + 1) * 128,
                                  i * SC:(i + 1) * SC], cs[:])
            ar_out = ar_outs[2 * l]
            nc.gpsimd.collective_compute(
                "AllReduce", mybir.AluOpType.add,
                replica_groups=[list(range(NC))],
                ins=[ar_in.opt()], outs=[ar_out.opt()])
            for e in range(ET):
                ld = pio.tile([128, S], F16, name=f"arl_{l}_{e}", tag="arl",
                              bufs=2)
                nc.sync.dma_start(ld[:], ar_out[e * 128:(e + 1) * 128, :])
                nc.vector.tensor_add(x_t[e][:], x_t[e][:], ld[:])

            # ---- FFN ----------------------------------------------------
            R2, rT2 = rms_norm_factors(l, "f")
            m_sb = []
            with tc.tile_pool(name=f"ps_gu_{l}", bufs=3, space="PSUM") as pgu:
                for fm in range(FT + 1):
                    rows = 128 if fm < FT else FPC - FT * 128
                    # per-fm weight slices [E, rows] -> [128, ET*rows]
                    wgs = pw.tile([128, ET * rows], F16, name=f"wgs{fm}",
                                  tag="wgs", bufs=2)
                    nc.sync.dma_start(
                        wgs[:].rearrange("p (t m) -> p t m", t=ET),
                        wg[l][:, fm * 128:fm * 128 + rows].rearrange(
                            "(t p) m -> p t m", p=128))
                    wus = pw.tile([128, ET * rows], F16, name=f"wus{fm}",
                                  tag="wus", bufs=2)
                    nc.sync.dma_start(
                        wus[:].rearrange("p (t m) -> p t m", t=ET),
                        wu[l][:, fm * 128:fm * 128 + rows].rearrange(
                            "(t p) m -> p t m", p=128))
                    mt = pff.tile([128, S], F16, name=f"m_{l}_{fm}",
                                  tag=f"mff{fm}", bufs=1)
                    for i in range(NSC):
                        c = slice(i * SC, (i + 1) * SC)
                        gp = pgu.tile([128, SC], F32, name=f"g{i}", tag="gps")
                        up = pgu.tile([128, SC], F32, name=f"u{i}", tag="ups")
                        for e in range(ET):
                            nc.tensor.matmul(
                                gp[0:rows, :],
                                wgs[:, e * rows:(e + 1) * rows],
                                x_t[e][:, c],
                                start=(e == 0), stop=(e == ET - 1))
                        for e in range(ET):
                            nc.tensor.matmul(
                                up[0:rows, :],
                                wus[:, e * rows:(e + 1) * rows],
                                x_t[e][:, c],
                                start=(e == 0), stop=(e == ET - 1))
                        gs = pff.tile([128, SC], F16, name=f"gs{i}", tag="gs",
                                      bufs=2)
                        us = pff.tile([128, SC], F16, name=f"us{i}", tag="us",
                                      bufs=2)
                        nc.vector.tensor_mul(gs[0:rows, :], gp[0:rows, :],
                                             R2[0:rows, c])
                        nc.scalar.activation(gs[0:rows, :], gs[0:rows, :],
                                             AF.Silu)
                        nc.vector.tensor_mul(us[0:rows, :], up[0:rows, :],
                                             R2[0:rows, c])
                        nc.vector.tensor_mul(mt[0:rows, c], gs[0:rows, :],
                                             us[0:rows, :])
                    m_sb.append(mt)

            with tc.tile_pool(name=f"ps_dn_{l}", bufs=4, space="PSUM") as pdn:
                for m in range(ET):
                    # wd slice [FPC, 128] -> [128, 6*128], fm-tile major
                    wds = pw.tile([128, (FT + 1) * 128], F16,
                                  name=f"wds{m}", tag="wds", bufs=2)
                    nc.sync.dma_start(
                        wds[:, 0:FT * 128].rearrange("p (t m) -> p t m",
                                                     t=FT),
                        wd[l][0:FT * 128, m * 128:(m + 1) * 128].rearrange(
                            "(t p) m -> p t m", p=128))
                    nc.sync.dma_start(
                        wds[0:FPC - FT * 128, FT * 128:(FT + 1) * 128],
                        wd[l][FT * 128:FPC, m * 128:(m + 1) * 128])
                    for i in range(NSC):
                        ps = pdn.tile([128, SC], F32, name=f"dn{i}",
                                      tag="dnps")
                        for fm in range(FT + 1):
                            rows = 128 if fm < FT else FPC - FT * 128
                            nc.tensor.matmul(
                                ps[:],
                                wds[0:rows, fm * 128:(fm + 1) * 128],
                                m_sb[fm][0:rows, i * SC:(i + 1) * SC],
                                start=(fm == 0), stop=(fm == FT))
                        cs = pio.tile([128, SC], F16, name=f"dnc_{m}_{i}",
                                      tag="dncast", bufs=2)
                        nc.vector.tensor_copy(cs[:], ps[:])
                        nc.sync.dma_start(
                            ar_in[m * 128:(m + 1) * 128,
                                  i * SC:(i + 1) * SC], cs[:])
            ar_out = ar_outs[2 * l + 1]
            nc.gpsimd.collective_compute(
                "AllReduce", mybir.AluOpType.add,
                replica_groups=[list(range(NC))],
                ins=[ar_in.opt()], outs=[ar_out.opt()])
            for e in range(ET):
                ld = pio.tile([128, S], F16, name=f"arf_{l}_{e}", tag="arl",
                              bufs=2)
                nc.sync.dma_start(ld[:], ar_out[e * 128:(e + 1) * 128, :])
                nc.vector.tensor_add(x_t[e][:], x_t[e][:], ld[:])

    nc.compile()
    return nc


def _host_prep(inputs):
    """Fold norms into weights, build tables, TP-shard -> per-core in_maps."""
    ids = np.asarray(inputs["input_ids"]).reshape(-1)
    x0 = np.asarray(inputs["embed"])[ids]          # [S, E] fp32
    x0T = np.ascontiguousarray(x0.T).astype(np.float16)

    ln1 = np.asarray(inputs["ln1"], dtype=np.float32)   # [L, E]
    ln2 = np.asarray(inputs["ln2"], dtype=np.float32)
    wq_f = ln1[:, :, None] * np.asarray(inputs["Wq"])   # [L, E, H*HD]
    wk_f = ln1[:, :, None] * np.asarray(inputs["Wk"])
    wv_f = ln1[:, :, None] * np.asarray(inputs["Wv"])
    wg_f = ln2[:, :, None] * np.asarray(inputs["Wg"])
    wu_f = ln2[:, :, None] * np.asarray(inputs["Wu"])
    wo_f = np.asarray(inputs["Wo"])                     # [L, H*HD, E]
    wd_f = np.asarray(inputs["Wd"])                     # [L, FF, E]

    inv = 1.0 / (ROPE_THETA ** (np.arange(0, HD, 2, dtype=np.float32) / HD))
    t = np.arange(S, dtype=np.float32)
    freqs = np.outer(t, inv)                       # [S, HD/2]
    emb = np.concatenate([freqs, freqs], axis=-1)  # [S, HD]
    cosT = np.ascontiguousarray(np.cos(emb).T).astype(np.float16)
    sinT = np.ascontiguousarray(np.sin(emb).T).astype(np.float16)

    rotP = np.zeros((HD, HD), dtype=np.float16)
    half = HD // 2
    for d in range(half):
        rotP[d + half, d] = -1.0
    for d in range(half, HD):
        rotP[d - half, d] = 1.0

    triM = np.triu(np.ones((128, 128), dtype=np.float16))   # [k, q] valid
    idnt = np.eye(128, dtype=np.float16)

    in_maps = []
    for c in range(NC):
        ds = slice(c * DPC, (c + 1) * DPC)
        fs = slice(c * FPC, (c + 1) * FPC)
        in_maps.append({
            "x0T": x0T,
            "wq": np.ascontiguousarray(wq_f[:, :, ds]).astype(np.float16),
            "wk": np.ascontiguousarray(wk_f[:, :, ds]).astype(np.float16),
            "wv": np.ascontiguousarray(wv_f[:, :, ds]).astype(np.float16),
            "wo": np.ascontiguousarray(wo_f[:, ds, :]).astype(np.float16),
            "wg": np.ascontiguousarray(wg_f[:, :, fs]).astype(np.float16),
            "wu": np.ascontiguousarray(wu_f[:, :, fs]).astype(np.float16),
            "wd": np.ascontiguousarray(wd_f[:, fs, :]).astype(np.float16),
            "cosT": cosT, "sinT": sinT, "rotP": rotP,
            "triM": triM, "idnt": idnt,
        })
    return in_maps


def kernel(**inputs):
    if "nc" not in _CACHE:
        _CACHE["nc"] = build_kernel()
    nc = _CACHE["nc"]
    in_maps = _host_prep(inputs)
    trace = os.environ.get("KERNEL_TRACE") == "1"
    res = run_bass_kernel_spmd(nc, in_maps, core_ids=list(range(NC)),
                               trace=trace)
    if trace and res.exec_time_ns is not None:
        print(f"HW exec time: {res.exec_time_ns} ns")
        _CACHE["exec_time_ns"] = res.exec_time_ns
        if res.instructions_and_trace:
            print("trace:", res.instructions_and_trace[1])

    out = np.zeros((2, L, B, H, S, HD), dtype=np.float32)
    for c in range(NC):
        kv = res.results[c]["kv_out"].astype(np.float32)  # [2, L, HPC, S, HD]
        for h in range(HPC):
            out[:, :, 0, c * HPC + h] = kv[:, :, h]
    return out

